# revision 3
# baseline (speedup 1.0000x reference)
"""v1.5: v1 structure + host diag/mse + optional DVE fold + multi-quadrant PE chains.

Same sharding as v1 (rows/cols interleaved idx%8==core), same ScalarE K-split.
Changes:
  * diagonal blocks + MSE moved to host (1.6% + O(n) of the work).
  * DVE prod tiles optionally folded in half (tensor_tensor add, bf16 2x)
    before PE reduction -- trades DVE cycles for PE columns.
  * PE reduction runs `pe_ways` independent accumulation chains into separate
    PSUM quadrants (tile_position col groups), exploiting PE array tiling
    concurrency; host sums the [ways, 512] output.
"""

import sys

sys.path.insert(0, "/opt/trn_rl_repo")

from contextlib import ExitStack

import numpy as np
import ml_dtypes

import concourse.bass as bass
import concourse.tile as tile
from concourse import mybir
from concourse.bacc import Bacc
from concourse.bass_utils import run_bass_kernel_spmd

ALPHA = 0.5
N = 8192
NCORES = 8
P = 128
B = N // P
NPC = N // NCORES
F32 = mybir.dt.float32
BF16 = mybir.dt.bfloat16
BF16NP = ml_dtypes.bfloat16

_CACHE = {}

# tunables (promoted into kernel.py once A/B-validated)
K = 10           # col-blocks on ScalarE
FOLD_MIN = 10_000  # fold prod tiles with ext >= this (10_000 = no folding)
PE_WAYS = 3      # independent PSUM quadrant accumulation chains (quadrant 3 has a HW bug)


def _build_nc(reps=1, k_sce=None, fold_min=None, pe_ways=None,
              skip_act=False, skip_dve=False, skip_bcast=False, skip_pe=False):
    k_sce = K if k_sce is None else k_sce
    fold_min = FOLD_MIN if fold_min is None else fold_min
    pe_ways = PE_WAYS if pe_ways is None else pe_ways

    nc = Bacc()
    t_c = nc.declare_dram_parameter("t_c", [1, NPC], BF16, isOutput=False)
    g_c = nc.declare_dram_parameter("g_c", [1, NPC], BF16, isOutput=False)
    grows = nc.declare_dram_parameter("g_rows", [P, B], F32, isOutput=False)
    tcols = nc.declare_dram_parameter("t_cols", [P, k_sce], F32, isOutput=False)
    out_acc = nc.declare_dram_parameter("out_acc", [P, k_sce], F32, isOutput=True)
    out_pe = nc.declare_dram_parameter("out_pe", [4, 512], F32, isOutput=True)

    relu = mybir.ActivationFunctionType.Relu
    alu = mybir.AluOpType

    with ExitStack() as ctx:
        tc = ctx.enter_context(tile.TileContext(nc))
        const = ctx.enter_context(tc.tile_pool(name="const", bufs=1))
        prods = ctx.enter_context(tc.tile_pool(name="prods", bufs=6))
        folds = ctx.enter_context(tc.tile_pool(name="folds", bufs=4))
        psum = ctx.enter_context(tc.tile_pool(name="psum", bufs=1, space="PSUM"))

        grows_s = const.tile([P, B], F32)
        tcols_s = const.tile([P, k_sce], F32)
        nc.scalar.dma_start(out=grows_s, in_=grows[:, :])
        nc.scalar.dma_start(out=tcols_s, in_=tcols[:, :])

        ones_col = const.tile([P, 1], BF16)
        nc.gpsimd.memset(ones_col, 1.0)
        warm = const.tile([P, 1], BF16)
        nc.gpsimd.memset(warm, 0.0)
        warm2 = const.tile([P, 1], BF16)
        nc.scalar.activation(out=warm2, in_=warm, func=relu, bias=0.0, scale=1.0)

        gbig_p = const.tile([P, NPC], BF16)
        tbig_s = const.tile([P, NPC], BF16)
        acc_s = const.tile([P, k_sce], F32)
        scr_a = const.tile([P, NPC], BF16)
        pe_all = psum.tile([P, 512], F32)
        nc.vector.memset(acc_s, 0.0)

        if not skip_bcast:
            nc.sync.dma_start(
                out=tbig_s[:, :256], in_=t_c[:, :256].to_broadcast([P, 256])
            )
            for h in range(0, NPC, 512):
                nc.sync.dma_start(
                    out=gbig_p[:, h : h + 512],
                    in_=g_c[:, h : h + 512].to_broadcast([P, 512]),
                )
            for h in range(256, NPC, 256):
                nc.sync.dma_start(
                    out=tbig_s[:, h : h + 256],
                    in_=t_c[:, h : h + 256].to_broadcast([P, 256]),
                )

        def emit_compute():
            # -- ScalarE: col-blocks q < K (suffix rows, fused accum) -----
            for q in range(k_sce) if not skip_act else []:
                lo = 16 * (q + 1)
                nc.scalar.activation(
                    out=scr_a[:, : NPC - lo],
                    in_=gbig_p[:, lo:NPC],
                    func=relu,
                    bias=tcols_s[:, q : q + 1],
                    scale=-1.0,
                    accum_out=acc_s[:, q : q + 1],
                )

            # -- VectorE + PE: row-blocks b > K over cols [16K, 16b) ------
            # pe chains: round-robin 512-col pieces across `pe_ways` PSUM
            # quadrant accumulators (independent array tiles).
            bs = list(range(k_sce + 1, B)) if not skip_dve else []
            widths = []
            for b in bs:
                ext = 16 * (b - k_sce)
                w = ext // 2 if ext >= fold_min else ext
                widths.append(w)
            n_pieces = sum(-(-w // 512) for w in widths) if not skip_pe else 0
            chain_started = [False] * pe_ways
            chain_i = [0]

            def emit_piece(ap, ww):
                t = chain_i[0] % pe_ways
                last = chain_i[0] >= n_pieces - pe_ways  # final round-robin lap
                chain_i[0] += 1
                nc.tensor.matmul(
                    pe_all[32 * t : 32 * t + 1, :ww],
                    lhsT=ones_col,
                    rhs=ap,
                    start=not chain_started[t],
                    stop=last,
                )
                chain_started[t] = True

            for b in bs:
                ext = 16 * (b - k_sce)
                prod = prods.tile([P, ext], BF16, tag="prod")
                nc.vector.tensor_scalar(
                    out=prod[:, :ext],
                    in0=tbig_s[:, 16 * k_sce : 16 * b],
                    scalar1=grows_s[:, b : b + 1],
                    scalar2=0.0,
                    op0=alu.subtract,
                    op1=alu.max,
                )
                if ext >= fold_min:
                    h = ext // 2
                    fol = folds.tile([P, h], BF16, tag="fold")
                    nc.vector.tensor_tensor(
                        out=fol[:, :h],
                        in0=prod[:, :h],
                        in1=prod[:, h:ext],
                        op=alu.add,
                    )
                    src_ap, w = fol, h
                else:
                    src_ap, w = prod, ext
                if not skip_pe:
                    for off in range(0, w, 512):
                        ww = min(512, w - off)
                        emit_piece(src_ap[:, off : off + ww], ww)

        if reps > 1:
            with tc.For_i(0, reps, 1):
                emit_compute()
        else:
            emit_compute()

        # ---- outputs ----------------------------------------------------
        # engines cannot move data across partitions: stage each PSUM quadrant
        # row at its own partition, then let the output DMA gather the strided
        # rows into the [4, 512] DRAM tensor.
        pe_stage = const.tile([P, 512], F32)
        nc.vector.memset(pe_stage, 0.0)
        if not (skip_dve or skip_pe):
            for t in range(pe_ways):
                nc.scalar.copy(
                    out=pe_stage[32 * t : 32 * t + 1, :],
                    in_=pe_all[32 * t : 32 * t + 1, :],
                )
        nc.sync.dma_start(out=out_acc[:, :], in_=acc_s)
        nc.sync.dma_start(out=out_pe[:, :], in_=pe_stage[0:97:32, :])

    nc.finalize()
    return nc


def _host_prep(logits, labels, k_sce=None):
    k_sce = K if k_sce is None else k_sce
    logits = np.asarray(logits, dtype=np.float32).reshape(N)
    labels = np.asarray(labels, dtype=np.float32).reshape(N)
    order = np.argsort(labels, kind="stable")
    g = np.ascontiguousarray(logits[order]).astype(np.float32)
    labs = labels[order]
    T = (1.0 + g).astype(np.float32)

    num_pairs = N * (N - 1) // 2
    tie_corr = 0.0
    change = np.nonzero(np.diff(labs))[0] + 1
    starts = np.concatenate([[0], change])
    ends = np.concatenate([change, [N]])
    for a, e in zip(starts, ends):
        m = int(e - a)
        if m > 1:
            num_pairs -= m * (m - 1) // 2
            gg = g[a:e].astype(np.float64)
            d = 1.0 + gg[None, :] - gg[:, None]
            tie_corr += float(np.maximum(d, 0.0)[np.tril_indices(m, -1)].sum())

    # host-side diagonal blocks: strict lower triangle within each 128-block
    G = g.reshape(B, P).astype(np.float64)
    D = 1.0 + G[:, None, :] - G[:, :, None]          # [B, a, b] = 1+g_b-g_a
    il = np.tril_indices(P, -1)
    diag_sum = float(np.maximum(D[:, il[0], il[1]], 0.0).sum())

    mse = float(np.mean((logits.astype(np.float64) - labels.astype(np.float64)) ** 2))

    grows = np.ascontiguousarray(g.reshape(B, P).T)
    tcols = np.ascontiguousarray(T.reshape(B, P).T[:, :k_sce])

    in_maps = []
    for c in range(NCORES):
        in_maps.append(
            {
                "t_c": np.ascontiguousarray(T[c::NCORES]).reshape(1, NPC).astype(BF16NP),
                "g_c": np.ascontiguousarray(g[c::NCORES]).reshape(1, NPC).astype(BF16NP),
                "g_rows": grows,
                "t_cols": tcols,
            }
        )
    return in_maps, (num_pairs, tie_corr, diag_sum, mse)


def _combine(results, host_terms):
    num_pairs, tie_corr, diag_sum, mse = host_terms
    rank_dev = 0.0
    for c in range(NCORES):
        rank_dev += results[c]["out_acc"].astype(np.float64).sum()
        rank_dev += results[c]["out_pe"].astype(np.float64).sum()
    rank_sum = rank_dev + diag_sum - tie_corr
    ranking = rank_sum / max(num_pairs, 1) if num_pairs > 0 else 0.0
    return np.float32(ALPHA * mse + (1.0 - ALPHA) * ranking)


def kernel(logits, labels, **_unused):
    in_maps, host_terms = _host_prep(logits, labels)
    if "nc" not in _CACHE:
        _CACHE["nc"] = _build_nc()
    run_bass_kernel_spmd(_CACHE["nc"], in_maps, list(range(NCORES)))
    res = run_bass_kernel_spmd(_CACHE["nc"], in_maps, list(range(NCORES)))
    return _combine(res.results, host_terms)


# revision 5
# speedup vs baseline: 1.5251x; 1.5251x over previous
"""v3: padded col-major full-width jobs + 3-chain PE reduce + host diag/mse/ties.

Strict-lower triangle at 128-block granularity = 63 col-block jobs; job for
col-block q covers rows 128(q+1)..8191 (width u=63-q units of 128 rows).
Cores own col-blocks in snake pairs (q, 63-q-style), 8 jobs/core, 252 units
each. SPMD needs identical shapes, so rank-k jobs are padded to the rank
maximum W = [63,55,47,39,31,23,15,7] (+28 units = 11% pad, filled with -BIG
so the hinge is 0). Per-core DRAM layout = 8 disjoint padded segments of -g
suffixes; one [1,35840] -> [128,35840] bf16 broadcast.

Each job is ONE instruction (vs 63 narrow ones in v1):
  * ScalarE ranks (param): relu(1*(-g) + t_col) with fused accum_out.
  * VectorE ranks: tensor_scalar((-g) - (-t_col), max 0), bf16 4x.
  * TensorE: ones-matmul partition-reduce of DVE tiles, 512-col pieces
    round-robined over 3 PSUM-quadrant accumulation chains (independent
    PE array tiles; quadrant 3 is unusable per HW bug).
"""

import sys

sys.path.insert(0, "/opt/trn_rl_repo")

from contextlib import ExitStack

import numpy as np
import ml_dtypes

import concourse.bass as bass
import concourse.tile as tile
from concourse import mybir
from concourse.bacc import Bacc
from concourse.bass_utils import run_bass_kernel_spmd

ALPHA = 0.5
N = 8192
NCORES = 8
P = 128
B = N // P
BIG_NEG = -1.0e30
F32 = mybir.dt.float32
BF16 = mybir.dt.bfloat16
BF16NP = ml_dtypes.bfloat16

W_RANK = [63, 55, 47, 39, 31, 23, 15, 7]   # padded widths per rank (units)
SEG_OFF = [128 * sum(W_RANK[:k]) for k in range(8)]   # segment offsets (cols)
L = 128 * sum(W_RANK)                                  # 35840

SCE_RANKS = (1, 6)   # ranks on ScalarE (sum 70 units); rest on DVE+PE
SCE_DELTA = 2        # units of ScE rank-1 tail handed to DVE (fine balance)
PE_WAYS = 3

_CACHE = {}


def _core_qs(c):
    """Rank-ordered col-blocks for core c (widths 63-q descending)."""
    return [c, 15 - c, 16 + c, 31 - c, 32 + c, 47 - c, 48 + c, 63 - c]


def _build_nc(reps=1, sce_ranks=None, pe_ways=None, sce_delta=None,
              skip_act=False, skip_dve=False, skip_bcast=False, skip_pe=False):
    sce_ranks = SCE_RANKS if sce_ranks is None else sce_ranks
    pe_ways = PE_WAYS if pe_ways is None else pe_ways
    sce_delta = SCE_DELTA if sce_delta is None else sce_delta
    dve_ranks = [k for k in range(8) if k not in sce_ranks]
    dve_ranks = dve_ranks[::-1]          # ascending width: fill PE fast
    # (rank, lo, hi) unit-ranges per engine; sce_delta units of the first ScE
    # rank's tail move to an extra DVE job for fine-grained balance.
    sce_jobs = [
        (k, 0, W_RANK[k] - (sce_delta if k == sce_ranks[0] else 0))
        for k in sce_ranks
    ]
    dve_jobs = [(k, 0, W_RANK[k]) for k in dve_ranks]
    if sce_delta:
        k0 = sce_ranks[0]
        dve_jobs.insert(0, (k0, W_RANK[k0] - sce_delta, W_RANK[k0]))

    nc = Bacc()
    gsrc = nc.declare_dram_parameter("gsrc", [1, L], BF16, isOutput=False)
    tq = nc.declare_dram_parameter("tq", [P, 8], F32, isOutput=False)
    tn = nc.declare_dram_parameter("tn", [P, 8], F32, isOutput=False)
    out_acc = nc.declare_dram_parameter("out_acc", [P, 8], F32, isOutput=True)
    out_pe = nc.declare_dram_parameter("out_pe", [4, 512], F32, isOutput=True)

    relu = mybir.ActivationFunctionType.Relu
    alu = mybir.AluOpType

    with ExitStack() as ctx:
        tc = ctx.enter_context(tile.TileContext(nc))
        const = ctx.enter_context(tc.tile_pool(name="const", bufs=1))
        prods = ctx.enter_context(tc.tile_pool(name="prods", bufs=3))
        psum = ctx.enter_context(tc.tile_pool(name="psum", bufs=1, space="PSUM"))

        tq_s = const.tile([P, 8], F32)
        tn_s = const.tile([P, 8], F32)
        nc.scalar.dma_start(out=tq_s, in_=tq[:, :])
        nc.scalar.dma_start(out=tn_s, in_=tn[:, :])

        ones_col = const.tile([P, 1], BF16)
        nc.gpsimd.memset(ones_col, 1.0)
        warm = const.tile([P, 1], BF16)
        nc.gpsimd.memset(warm, 0.0)
        warm2 = const.tile([P, 1], BF16)
        nc.scalar.activation(out=warm2, in_=warm, func=relu, bias=0.0, scale=1.0)

        gsrc_bc = const.tile([P, L], BF16)
        acc_s = const.tile([P, 8], F32)
        scr_a = const.tile([P, 128 * 63], BF16)
        pe_all = psum.tile([P, 512], F32)
        nc.vector.memset(acc_s, 0.0)

        # broadcast the padded segments, smallest-rank (last consumed first
        # by the ascending-width DVE order) first; round-robin over queues.
        if not skip_bcast:
            queues = [nc.sync, nc.scalar, nc.gpsimd]
            qi = 0
            for k in range(7, -1, -1):
                off, w = SEG_OFF[k], 128 * W_RANK[k]
                for h in range(0, w, 2048):
                    hw = min(2048, w - h)
                    queues[qi % len(queues)].dma_start(
                        out=gsrc_bc[:, off + h : off + h + hw],
                        in_=gsrc[:, off + h : off + h + hw].to_broadcast([P, hw]),
                    )
                    qi += 1

        def emit_compute():
            # -- ScalarE jobs ---------------------------------------------
            for k, lo_u, hi_u in sce_jobs if not skip_act else []:
                lo, w = 128 * lo_u, 128 * (hi_u - lo_u)
                nc.scalar.activation(
                    out=scr_a[:, :w],
                    in_=gsrc_bc[:, SEG_OFF[k] + lo : SEG_OFF[k] + lo + w],
                    func=relu,
                    bias=tq_s[:, k : k + 1],
                    scale=1.0,
                    accum_out=acc_s[:, k : k + 1],
                )

            # -- VectorE + PE jobs ----------------------------------------
            n_pieces = sum(
                -(-128 * (hi - lo) // 512) for _, lo, hi in dve_jobs
            )
            chain_started = [False] * pe_ways
            ci = [0]

            def emit_piece(ap, ww):
                t = ci[0] % pe_ways
                last = ci[0] >= n_pieces - pe_ways
                ci[0] += 1
                nc.tensor.matmul(
                    pe_all[32 * t : 32 * t + 1, :ww],
                    lhsT=ones_col,
                    rhs=ap,
                    start=not chain_started[t],
                    stop=last,
                )
                chain_started[t] = True

            for k, lo_u, hi_u in dve_jobs if not skip_dve else []:
                lo, w = 128 * lo_u, 128 * (hi_u - lo_u)
                prod = prods.tile([P, 128 * 63], BF16, tag="prod")
                nc.vector.tensor_scalar(
                    out=prod[:, :w],
                    in0=gsrc_bc[:, SEG_OFF[k] + lo : SEG_OFF[k] + lo + w],
                    scalar1=tn_s[:, k : k + 1],
                    scalar2=0.0,
                    op0=alu.subtract,
                    op1=alu.max,
                )
                if not skip_pe:
                    for off in range(0, w, 512):
                        ww = min(512, w - off)
                        emit_piece(prod[:, off : off + ww], ww)

        if reps > 1:
            with tc.For_i(0, reps, 1):
                emit_compute()
        else:
            emit_compute()

        # ---- outputs ----------------------------------------------------
        pe_stage = const.tile([P, 512], F32)
        nc.vector.memset(pe_stage, 0.0)
        if not (skip_dve or skip_pe):
            for t in range(pe_ways):
                nc.scalar.copy(
                    out=pe_stage[32 * t : 32 * t + 1, :],
                    in_=pe_all[32 * t : 32 * t + 1, :],
                )
        nc.sync.dma_start(out=out_acc[:, :], in_=acc_s)
        nc.sync.dma_start(out=out_pe[:, :], in_=pe_stage[0:97:32, :])

    nc.finalize()
    return nc


def _host_prep(logits, labels):
    logits = np.asarray(logits, dtype=np.float32).reshape(N)
    labels = np.asarray(labels, dtype=np.float32).reshape(N)
    order = np.argsort(labels, kind="stable")
    g = np.ascontiguousarray(logits[order]).astype(np.float32)
    labs = labels[order]
    T = (1.0 + g).astype(np.float32)

    num_pairs = N * (N - 1) // 2
    tie_corr = 0.0
    change = np.nonzero(np.diff(labs))[0] + 1
    starts = np.concatenate([[0], change])
    ends = np.concatenate([change, [N]])
    for a, e in zip(starts, ends):
        m = int(e - a)
        if m > 1:
            num_pairs -= m * (m - 1) // 2
            gg = g[a:e].astype(np.float64)
            d = 1.0 + gg[None, :] - gg[:, None]
            tie_corr += float(np.maximum(d, 0.0)[np.tril_indices(m, -1)].sum())

    G = g.reshape(B, P).astype(np.float64)
    D = 1.0 + G[:, None, :] - G[:, :, None]
    il = np.tril_indices(P, -1)
    diag_sum = float(np.maximum(D[:, il[0], il[1]], 0.0).sum())
    mse = float(np.mean((logits.astype(np.float64) - labels.astype(np.float64)) ** 2))

    gneg = (-g).astype(np.float32)
    in_maps = []
    for c in range(NCORES):
        qs = _core_qs(c)
        gs = np.full(L, BIG_NEG, dtype=np.float32)
        tqv = np.zeros((P, 8), dtype=np.float32)
        tnv = np.zeros((P, 8), dtype=np.float32)
        for k, q in enumerate(qs):
            u = 63 - q
            if u > 0:
                gs[SEG_OFF[k] : SEG_OFF[k] + 128 * u] = gneg[128 * (q + 1) :]
            tqv[:, k] = T[128 * q : 128 * (q + 1)]
            tnv[:, k] = -tqv[:, k]
        in_maps.append(
            {
                "gsrc": gs.reshape(1, L).astype(BF16NP),
                "tq": tqv,
                "tn": tnv,
            }
        )
    return in_maps, (num_pairs, tie_corr, diag_sum, mse)


def _combine(results, host_terms):
    num_pairs, tie_corr, diag_sum, mse = host_terms
    rank_dev = 0.0
    for c in range(NCORES):
        rank_dev += results[c]["out_acc"].astype(np.float64).sum()
        rank_dev += results[c]["out_pe"].astype(np.float64).sum()
    rank_sum = rank_dev + diag_sum - tie_corr
    ranking = rank_sum / max(num_pairs, 1) if num_pairs > 0 else 0.0
    return np.float32(ALPHA * mse + (1.0 - ALPHA) * ranking)


def kernel(logits, labels, **_unused):
    in_maps, host_terms = _host_prep(logits, labels)
    if "nc" not in _CACHE:
        _CACHE["nc"] = _build_nc()
    run_bass_kernel_spmd(_CACHE["nc"], in_maps, list(range(NCORES)))
    res = run_bass_kernel_spmd(_CACHE["nc"], in_maps, list(range(NCORES)))
    return _combine(res.results, host_terms)
